# revision 1
# baseline (speedup 1.0000x reference)
"""Trainium2 Bass kernel for single-head 2D attention (B=16, C=512, H=W=32).

Data-parallel over batch: 16 batch items / 8 cores = 2 per core. Weights
replicated. All matmuls run in float32r (full PE rate); layouts are chosen
so no on-device transpose is ever needed:

  per batch item b (x_cn = x[b] viewed as [C, N=1024], channel-major):
    Qt[o,n] = sum_c wqT[c,o] x[c,n] + bq[o]      (lhsT=wqT, rhs=x)
    Kt[o,n] = likewise
    V[n,o]  = sum_c x[c,n] wvT[c,o]              (lhsT=x,   rhs=wvT)
    St[j,i] = sum_o Kt[o,j] Qt[o,i]              (lhsT=Kt,  rhs=Qt)
    E[j,i]  = exp(St[j,i] / sqrt(C))             (ACT, no max-subtract:
                                                  scores are O(+-6))
    den[*,i]= sum_j E[j,i]   via all-ones lhsT   (sum over partitions AND
                                                  broadcast to 128 parts)
    svT[c,i]= (sum_j V[j,c] E[j,i]) * recip[i]   (lhsT=V, rhs=E)
    y[c',n] = x[c',n] + sum_c woT[c,c'] svT[c,n] + bo_eff[c']
  with bo_eff = bo + wo @ bv (V bias folded in on the host).
"""

import math

import numpy as np

import concourse.bass as bass
import concourse.mybir as mybir
import concourse.tile as tile
from concourse import bacc, bass_utils

B, C, H, W = 16, 512, 32, 32
N = H * W           # 1024 tokens
NCORES = 8
BPC = B // NCORES   # batch items per core
P = 128
CO = C // P         # 4 channel chunks
NB = N // 512       # 2 psum-bank slices of the token dim
NT = N // P         # 8 token chunks

_CACHE: dict = {}


def _build():
    f32 = mybir.dt.float32
    f32r = mybir.dt.float32r
    Ident = mybir.ActivationFunctionType.Identity
    Exp = mybir.ActivationFunctionType.Exp
    add = mybir.AluOpType.add

    nc = bacc.Bacc("TRN2", debug=False, enable_asserts=False, num_devices=NCORES)
    x_d = nc.dram_tensor("x", (BPC, C, N), f32r, kind="ExternalInput").ap()
    w_d = {
        k: nc.dram_tensor(f"w{k}t", (C, C), f32r, kind="ExternalInput").ap()
        for k in ("q", "k", "v", "o")
    }
    bq_d = nc.dram_tensor("bq", (P, CO), f32, kind="ExternalInput").ap()
    bk_d = nc.dram_tensor("bk", (P, CO), f32, kind="ExternalInput").ap()
    bo_d = nc.dram_tensor("bo", (P, CO), f32, kind="ExternalInput").ap()
    ones_d = nc.dram_tensor("ones", (P, P), f32r, kind="ExternalInput").ap()
    y_d = nc.dram_tensor("y", (BPC, C, N), f32, kind="ExternalOutput").ap()

    with tile.TileContext(nc) as tc:
        with (
            tc.tile_pool(name="wp", bufs=1) as wp,
            tc.tile_pool(name="xp", bufs=2) as xp,
            tc.tile_pool(name="qkp", bufs=1) as qkp,
            tc.tile_pool(name="vp", bufs=2) as vp,
            tc.tile_pool(name="ep", bufs=1) as ep,
            tc.tile_pool(name="svp", bufs=1) as svp,
            tc.tile_pool(name="rp", bufs=1) as rp,
            tc.tile_pool(name="yp", bufs=4) as yp,
            tc.tile_pool(name="ps", bufs=4, space="PSUM") as ps,
        ):
            wt = {}
            for k in ("q", "k", "v", "o"):
                t = wp.tile([P, CO, C], f32r, tag=f"w{k}")
                nc.sync.dma_start(t[:], w_d[k].rearrange("(co p) o -> p co o", p=P))
                wt[k] = t
            bq_t = wp.tile([P, CO], f32, tag="bq")
            nc.sync.dma_start(bq_t[:], bq_d)
            bk_t = wp.tile([P, CO], f32, tag="bk")
            nc.sync.dma_start(bk_t[:], bk_d)
            bo_t = wp.tile([P, CO], f32, tag="bo")
            nc.sync.dma_start(bo_t[:], bo_d)
            ones_t = wp.tile([P, P], f32r, tag="ones")
            nc.sync.dma_start(ones_t[:], ones_d)

            inv_sqrt_c = 1.0 / math.sqrt(C)

            for b in range(BPC):
                x_sb = xp.tile([P, CO, N], f32r, tag="x")
                nc.sync.dma_start(x_sb[:], x_d[b].rearrange("(ci p) n -> p ci n", p=P))

                # --- Qt / Kt projections (channel-major) ---
                qt = qkp.tile([P, CO, N], f32r, tag="qt")
                kt = qkp.tile([P, CO, N], f32r, tag="kt")
                for t_sb, w_t, b_t in ((qt, wt["q"], bq_t), (kt, wt["k"], bk_t)):
                    for oc in range(CO):
                        for nb in range(NB):
                            pt = ps.tile([P, 512], f32, tag="ps")
                            for ci in range(CO):
                                nc.tensor.matmul(
                                    pt[:],
                                    w_t[:, ci, oc * P:(oc + 1) * P],
                                    x_sb[:, ci, nb * 512:(nb + 1) * 512],
                                    start=(ci == 0), stop=(ci == CO - 1),
                                )
                            nc.scalar.activation(
                                t_sb[:, oc, nb * 512:(nb + 1) * 512], pt[:],
                                Ident, bias=b_t[:, oc:oc + 1],
                            )

                # --- V projection (token-major, bias folded into bo_eff) ---
                v_sb = vp.tile([P, NT, C], f32r, tag="v")
                for t8 in range(NT):
                    pt = ps.tile([P, 512], f32, tag="ps")
                    for ci in range(CO):
                        nc.tensor.matmul(
                            pt[:],
                            x_sb[:, ci, t8 * P:(t8 + 1) * P],
                            wt["v"][:, ci, :],
                            start=(ci == 0), stop=(ci == CO - 1),
                        )
                    nc.vector.tensor_copy(v_sb[:, t8, :], pt[:])

                # --- St = Kt^T Qt, then exp (scores scaled inside ACT) ---
                est = ep.tile([P, NT, N], f32r, tag="est")
                for jc in range(NT):
                    for ib in range(NB):
                        pt = ps.tile([P, 512], f32, tag="ps")
                        for oc in range(CO):
                            nc.tensor.matmul(
                                pt[:],
                                kt[:, oc, jc * P:(jc + 1) * P],
                                qt[:, oc, ib * 512:(ib + 1) * 512],
                                start=(oc == 0), stop=(oc == CO - 1),
                            )
                        nc.scalar.activation(
                            est[:, jc, ib * 512:(ib + 1) * 512], pt[:],
                            Exp, scale=inv_sqrt_c,
                        )

                # --- softmax denominators: all-ones lhsT sums over partitions
                #     and broadcasts the result to every partition ---
                recip = rp.tile([P, N], f32, tag="recip")
                for ib in range(NB):
                    pt = ps.tile([P, 512], f32, tag="ps")
                    for jc in range(NT):
                        nc.tensor.matmul(
                            pt[:], ones_t[:],
                            est[:, jc, ib * 512:(ib + 1) * 512],
                            start=(jc == 0), stop=(jc == NT - 1),
                        )
                    nc.vector.reciprocal(recip[:, ib * 512:(ib + 1) * 512], pt[:])

                # --- svT = (V^T E) * recip  (channel-major) ---
                sv = svp.tile([P, CO, N], f32r, tag="sv")
                for cc in range(CO):
                    for ib in range(NB):
                        pt = ps.tile([P, 512], f32, tag="ps")
                        for jc in range(NT):
                            nc.tensor.matmul(
                                pt[:],
                                v_sb[:, jc, cc * P:(cc + 1) * P],
                                est[:, jc, ib * 512:(ib + 1) * 512],
                                start=(jc == 0), stop=(jc == NT - 1),
                            )
                        nc.vector.tensor_mul(
                            sv[:, cc, ib * 512:(ib + 1) * 512], pt[:],
                            recip[:, ib * 512:(ib + 1) * 512],
                        )

                # --- output projection + bias + residual ---
                for c2 in range(CO):
                    for nb in range(NB):
                        pt = ps.tile([P, 512], f32, tag="ps")
                        for cc in range(CO):
                            nc.tensor.matmul(
                                pt[:],
                                wt["o"][:, cc, c2 * P:(c2 + 1) * P],
                                sv[:, cc, nb * 512:(nb + 1) * 512],
                                start=(cc == 0), stop=(cc == CO - 1),
                            )
                        yt = yp.tile([P, 512], f32, tag="y")
                        nc.vector.scalar_tensor_tensor(
                            yt[:], pt[:], bo_t[:, c2:c2 + 1],
                            x_sb[:, c2, nb * 512:(nb + 1) * 512].bitcast(f32),
                            add, add,
                        )
                        nc.sync.dma_start(
                            y_d[b, c2 * P:(c2 + 1) * P, nb * 512:(nb + 1) * 512],
                            yt[:],
                        )
    nc.compile()
    return nc


def _prep_inputs(inputs):
    x = np.asarray(inputs["x"], np.float32).reshape(B, C, N)
    wts = {}
    for k in ("q", "k", "v", "o"):
        wts[f"w{k}t"] = np.ascontiguousarray(np.asarray(inputs[f"w{k}"], np.float32).T)
    bq = np.asarray(inputs["bq"], np.float32)
    bk = np.asarray(inputs["bk"], np.float32)
    bv = np.asarray(inputs["bv"], np.float32)
    bo = np.asarray(inputs["bo"], np.float32)
    wo = np.asarray(inputs["wo"], np.float32)
    bo_eff = bo + wo @ bv

    def per_part(v):  # [C] -> [P, CO]
        return np.ascontiguousarray(v.reshape(CO, P).T)

    shared = {
        **wts,
        "bq": per_part(bq),
        "bk": per_part(bk),
        "bo": per_part(bo_eff),
        "ones": np.ones((P, P), np.float32),
    }
    in_maps = [
        {**shared, "x": np.ascontiguousarray(x[i * BPC:(i + 1) * BPC])}
        for i in range(NCORES)
    ]
    return in_maps


def kernel(**inputs) -> np.ndarray:
    if "nc" not in _CACHE:
        _CACHE["nc"] = _build()
    nc = _CACHE["nc"]
    in_maps = _prep_inputs(inputs)
    res = bass_utils.run_bass_kernel_spmd(nc, in_maps, core_ids=list(range(NCORES)))
    y = np.concatenate([r["y"] for r in res.results], axis=0)
    return y.reshape(B, C, H, W)
